# revision 1
# baseline (speedup 1.0000x reference)
"""Trainium2 Bass kernel for nn_BDHLanguageModel.

Model: single shared-state relu RNN over all B*T tokens (t-major order),
followed by a vocab-projection head.

    tokens = input_seq.T.reshape(-1)            # [T*B]
    xs     = E[tokens]                          # [T*B, D]
    v_{n+1} = relu(W_r @ v_n + xs[n] + b_r)     # strictly sequential
    logits  = vs @ head_w.T + head_b            # [T*B, V]
    out     = logits.reshape(T, B, V).transpose(1, 0, 2)

Distribution: the recurrence is replicated on all 8 cores (it is strictly
sequential, state is tiny); the head GEMM is tensor-parallel over vocab
(head_w row-sharded, 4000 rows per core). Each core writes its [B, T, 4000]
logit shard; the host concatenates along vocab.

Per-core device schedule:
  1. prep: DMA weights in; gather xs rows of E by token id (indirect DMA,
     128 rows/tile); PE-transpose each [128tok x 128d] tile so d lands on
     partitions; fold b_r in (DVE tensor_scalar) -> xb sbuf [128, 2048, 8] f32.
  2. recurrence (For_i hardware loop, 16 steps/iter): per step 64 self-loading
     [128x128] bf16 matmuls (W.T tiles stationary, v chunk [128,1] moving)
     accumulate z into a [128, 8] psum tile; DVE adds xb[n] and relus into the
     ping/pong v buffer (bf16); ACT copies v into the vs.T store [128, 2048, 8].
  3. head: for each 500-vocab chunk, stream head_w.T slab from DRAM, 16x8
     matmuls (vs.T tiles stationary, head slab moving, N=500), add head_b
     (replicated), DMA [128tok, 500] f32 to the [B, T, 4000] output with the
     token->(b,t) unshuffle done by the DMA access pattern.

bf16 numerics: host-sim of the 2048-step recurrence in bf16 (fp32 psum) gives
absmax-relative error ~4.5e-3 vs fp32 with no growth over steps.
"""

import numpy as np
import ml_dtypes

import concourse.bass as bass
import concourse.mybir as mybir
import concourse.tile as tile
from concourse import bacc
from concourse.bass import ds, ts
from concourse.bass_utils import run_bass_kernel_spmd
from concourse.masks import make_identity

BF16 = ml_dtypes.bfloat16

B, T, D, V = 4, 512, 1024, 32000
P = 128
DC = D // P            # 8 d-chunks
NT = B * T             # 2048 sequential steps
NTT = NT // P          # 16 token tiles
N_CORES = 8
VS = V // N_CORES      # 4000 vocab rows per core
VCH = 500              # vocab cols per head matmul (<=512 psum bank)
NVC = VS // VCH        # 8 vocab chunks per core
KU = 16                # recurrence steps per For_i iteration

_BUILD_CACHE = {}


def _build(n_tok_tiles=NTT, debug_dump=False):
    """Build the per-core Bass program (identical on every core)."""
    nt = n_tok_tiles * P

    nc = bacc.Bacc(None, target_bir_lowering=False, debug=False)

    f32 = mybir.dt.float32
    bf16 = mybir.dt.bfloat16
    i32 = mybir.dt.int32

    wt_d = nc.dram_tensor("wt", [P, DC * DC * P], bf16, kind="ExternalInput")
    tok_d = nc.dram_tensor("tok", [n_tok_tiles, P], i32, kind="ExternalInput")
    emb_d = nc.dram_tensor("emb", [V, D], f32, kind="ExternalInput")
    v0_d = nc.dram_tensor("v0m", [P, DC], bf16, kind="ExternalInput")
    b_d = nc.dram_tensor("bmat", [P, DC], f32, kind="ExternalInput")
    hwt_d = nc.dram_tensor("hwt", [NVC, DC, P, VCH], bf16, kind="ExternalInput")
    hbr_d = nc.dram_tensor("hbr", [P, VS], f32, kind="ExternalInput")
    out_d = nc.dram_tensor("out", [B, T, VS], f32, kind="ExternalOutput")
    if debug_dump:
        dbg_vst_d = nc.dram_tensor("dbg_vst", [P, DC, nt], f32, kind="ExternalOutput")
        dbg_xb_d = nc.dram_tensor("dbg_xb", [P, nt, DC], f32, kind="ExternalOutput")

    with tile.TileContext(nc) as tc:
        with (
            tc.tile_pool(name="const", bufs=1) as cpool,
            tc.tile_pool(name="gather", bufs=2) as gpool,
            tc.tile_pool(name="head", bufs=2) as hpool,
            tc.tile_pool(name="outp", bufs=3) as opool,
            tc.tile_pool(name="tps", bufs=2, space="PSUM") as tpsum,
            tc.tile_pool(name="zps", bufs=1, space="PSUM") as zpsum,
            tc.tile_pool(name="hps", bufs=2, space="PSUM") as hpsum,
        ):
            # ---- persistent SBUF state ----
            wt_sb = cpool.tile([P, DC * DC * P], bf16)      # W.T tiles (j,i)
            xb_sb = cpool.tile([P, nt, DC], f32)            # x_n + b_r, d on partitions
            vst_sb = cpool.tile([P, DC, nt], bf16)          # vs.T store (k-major)
            tok_sb = cpool.tile([P, n_tok_tiles], i32)
            b_sb = cpool.tile([P, DC], f32)
            vbuf_sb = cpool.tile([P, 2, DC], bf16)          # ping/pong state
            hbr_sb = cpool.tile([P, VS], f32)
            ident_sb = cpool.tile([P, P], f32)

            nc.sync.dma_start(wt_sb[:], wt_d[:])
            nc.sync.dma_start(tok_sb[:], tok_d[:].rearrange("m p -> p m"))
            nc.sync.dma_start(b_sb[:], b_d[:])
            nc.sync.dma_start(vbuf_sb[:, 0, :], v0_d[:])
            nc.sync.dma_start(hbr_sb[:], hbr_d[:])
            make_identity(nc, ident_sb[:])

            # ---- prep: gather E rows, transpose to d-on-partitions, + b_r ----
            for mt in range(n_tok_tiles):
                xs_sb = gpool.tile([P, D], f32, tag="xs")
                nc.gpsimd.indirect_dma_start(
                    out=xs_sb[:],
                    out_offset=None,
                    in_=emb_d[:],
                    in_offset=bass.IndirectOffsetOnAxis(
                        ap=tok_sb[:, mt : mt + 1], axis=0
                    ),
                )
                for j in range(DC):
                    tp = tpsum.tile([P, P], f32, tag="tp")
                    nc.tensor.transpose(tp[:], xs_sb[:, ts(j, P)], ident_sb[:])
                    nc.vector.tensor_scalar_add(
                        xb_sb[:, ts(mt, P), j], tp[:], b_sb[:, j : j + 1]
                    )

            # ---- recurrence ----
            zp0 = zpsum.tile([P, DC], f32, name="zp0")
            zp1 = zpsum.tile([P, DC], f32, name="zp1")
            zps = [zp0, zp1]

            def rec_step(n, s):
                vin = vbuf_sb[:, s % 2, :]
                vout = vbuf_sb[:, (s + 1) % 2, :]
                zp = zps[s % 2]
                for i in range(DC):
                    for j in range(DC):
                        nc.tensor.matmul(
                            zp[:, i : i + 1],
                            lhsT=wt_sb[:, ts(j * DC + i, P)],
                            rhs=vin[:, j : j + 1],
                            start=(j == 0),
                            stop=(j == DC - 1),
                        )
                nc.vector.tensor_tensor(
                    out=vout,
                    in0=zp[:],
                    in1=xb_sb[:, ds(n, 1), :],
                    op=mybir.AluOpType.add,
                )
                nc.vector.tensor_scalar_max(vout, vout, 0.0)
                # store b-major: head out tiles become contiguous (b, t) blocks
                col = (n % B) * (nt // B) + n // B
                nc.scalar.copy(vst_sb[:, :, ds(col, 1)], vout)

            for n_i in range(nt):
                rec_step(n_i, n_i)

            if debug_dump:
                vst_f32 = cpool.tile([P, DC, nt], f32)
                nc.vector.tensor_copy(vst_f32[:], vst_sb[:])
                nc.sync.dma_start(dbg_vst_d[:], vst_f32[:])
                nc.sync.dma_start(dbg_xb_d[:], xb_sb[:])

            # ---- head GEMM (vocab shard) ----
            for nv in range(NVC):
                hw_sb = hpool.tile([P, DC, VCH], bf16, tag="hw")
                nc.sync.dma_start(hw_sb[:], hwt_d[nv].rearrange("k p c -> p k c"))
                for mt in range(n_tok_tiles):
                    hp = hpsum.tile([P, 512], f32, tag="hp")
                    for k in range(DC):
                        nc.tensor.matmul(
                            hp[:, :VCH],
                            lhsT=vst_sb[:, k, ts(mt, P)],
                            rhs=hw_sb[:, k, :],
                            start=(k == 0),
                            stop=(k == DC - 1),
                        )
                    o_sb = opool.tile([P, VCH], f32, tag="o")
                    nc.vector.tensor_tensor(
                        out=o_sb[:],
                        in0=hp[:, :VCH],
                        in1=hbr_sb[:, ts(nv, VCH)],
                        op=mybir.AluOpType.add,
                    )
                    # vst col = b*(nt//B) + t; tile mt covers cols
                    # [128*mt, 128*mt+128) -> contiguous (b, t) segments
                    tpb = nt // B  # t-positions per b
                    c0 = P * mt
                    while c0 < P * (mt + 1):
                        b_idx = c0 // tpb
                        seg = min(P * (mt + 1), (b_idx + 1) * tpb) - c0
                        p0 = c0 - P * mt
                        nc.sync.dma_start(
                            out_d[b_idx, ds(c0 - b_idx * tpb, seg), ds(nv * VCH, VCH)],
                            o_sb[p0 : p0 + seg, :],
                        )
                        c0 += seg

    nc.compile()
    return nc


def _get_program(n_tok_tiles=NTT):
    if n_tok_tiles not in _BUILD_CACHE:
        _BUILD_CACHE[n_tok_tiles] = _build(n_tok_tiles)
    return _BUILD_CACHE[n_tok_tiles]


def _host_prep(input_seq, E, W_r, b_r, head_w, head_b, v0, n_tok_tiles=NTT):
    nt = n_tok_tiles * P
    tokens = (
        np.ascontiguousarray(np.asarray(input_seq).T)
        .reshape(-1)[:nt]
        .astype(np.int32)
        .reshape(n_tok_tiles, P)
    )
    W = np.asarray(W_r, dtype=np.float32)
    wt = np.ascontiguousarray(
        W.reshape(DC, P, DC, P).transpose(3, 2, 0, 1)
    ).reshape(P, DC * DC * P).astype(BF16)
    bmat = np.ascontiguousarray(np.asarray(b_r, np.float32).reshape(DC, P).T)
    v0m = np.ascontiguousarray(np.asarray(v0, np.float32).reshape(DC, P).T).astype(
        BF16
    )
    emb = np.ascontiguousarray(np.asarray(E, np.float32))
    hw = np.asarray(head_w, np.float32)
    hb = np.asarray(head_b, np.float32)

    in_maps = []
    for c in range(N_CORES):
        hw_c = hw[c * VS : (c + 1) * VS]  # [4000, 1024]
        hwt_c = np.ascontiguousarray(
            hw_c.reshape(NVC, VCH, DC, P).transpose(0, 2, 3, 1)
        ).astype(BF16)
        hbr_c = np.ascontiguousarray(
            np.broadcast_to(hb[c * VS : (c + 1) * VS][None, :], (P, VS))
        ).astype(np.float32)
        in_maps.append(
            {
                "wt": wt,
                "tok": tokens,
                "emb": emb,
                "v0m": v0m,
                "bmat": bmat,
                "hwt": hwt_c,
                "hbr": hbr_c,
            }
        )
    return in_maps


def run(inputs, n_tok_tiles=NTT, trace=False, tmpdir=None):
    """Run on hardware; returns (logits [B, T, V] f32, BassKernelResults)."""
    nc = _get_program(n_tok_tiles)
    in_maps = _host_prep(**inputs, n_tok_tiles=n_tok_tiles)
    br = run_bass_kernel_spmd(
        nc,
        in_maps,
        core_ids=list(range(N_CORES)),
        trace=trace,
        tmpdir=tmpdir,
    )
    shards = [r["out"] for r in br.results]  # each [B, T, VS]
    logits = np.concatenate(shards, axis=2)
    return logits, br


def kernel(input_seq, E, W_r, b_r, head_w, head_b, v0):
    inputs = dict(
        input_seq=input_seq, E=E, W_r=W_r, b_r=b_r,
        head_w=head_w, head_b=head_b, v0=v0,
    )
    logits, _ = run(inputs)
    return logits



# revision 2
# speedup vs baseline: 1.1978x; 1.1978x over previous
"""Trainium2 Bass kernel for nn_BDHLanguageModel.

Model: single shared-state relu RNN over all B*T tokens (t-major order),
followed by a vocab-projection head:

    tokens = input_seq.T.reshape(-1)            # [T*B]
    xs     = E[tokens]                          # [T*B, D]
    v_{n+1} = relu(W_r @ v_n + xs[n] + b_r)     # strictly sequential
    logits  = vs @ head_w.T + head_b            # [T*B, V]
    out     = logits.reshape(T, B, V).transpose(1, 0, 2)

Distribution: the recurrence is replicated on all 8 cores (strictly
sequential, tiny state); the head GEMM is tensor-parallel over vocab
(head_w row-sharded, 4000 rows per core). Each core writes its
[B, T, 4000] bf16 logit shard; the host concatenates along vocab, casts
to f32 and adds head_b.

Host-side prep (cheap, ~0.2s): embedding gather xs = E[tokens], b_r fold,
transpose to d-on-partitions, bf16 casts, W/head_w tile layouts. This
keeps the 131MB f32 embedding table off the per-core upload path (v1
replicated it to all 8 cores).

Device schedule per core:
  1. DMA in W.T tiles (2MB), xb = xs+b_r (4MB), v0; prefetch all 8
     head_w slabs (8MB) during the recurrence.
  2. recurrence: per step 64 self-loading [128x128] bf16 matmuls
     (W.T tiles stationary with FWL, v chunk [128,1] moving) accumulate
     z into a [128, 8] psum tile (ping/pong); DVE adds xb[n], then
     relus directly into the vs.T store column (b-major order). The
     next step's matmuls read the state from that column. Measured at
     ~3.5us/step = the FWL LDWEIGHTS stream floor (the whole 1Kx1K W
     must re-enter the PE array every step).
  3. head: per 500-vocab chunk x 128-token tile: 8 accumulating
     matmuls (vs.T tiles stationary, head slab moving, N=500); DVE
     copies psum to a bf16 tile; DMA to out[b, t0:t0+128, 500-chunk]
     (each token tile is one contiguous (b, t) block).

Execution: the Bass program runs on cores 0-7 through the same
bass2jax/PJRT path run_bass_kernel_spmd uses under axon, with the
shard_map wrapper jitted once and cached (run_bass_kernel_spmd rebuilds
it per call, costing ~12s/call of retrace), and with the donated zero
output buffers replaced by persistent device-resident arrays (the
kernel writes every output element, so nothing depends on the zeros;
this removes a 131MB/call upload). run_bass_kernel_spmd itself is used
for traced/profiled runs (see test.py).

bf16 numerics: absmax-relative error vs the f32 reference is 4.8e-3
(dominated by the bf16 recurrence; bf16 logits add ~1e-3).
"""

import numpy as np
import ml_dtypes

import concourse.bass as bass
import concourse.mybir as mybir
import concourse.tile as tile
from concourse import bacc
from concourse.bass import ds, ts
from concourse.bass_utils import run_bass_kernel_spmd

BF16 = ml_dtypes.bfloat16

B, T, D, V = 4, 512, 1024, 32000
P = 128
DC = D // P            # 8 d-chunks
NT = B * T             # 2048 sequential steps
NTT = NT // P          # 16 token tiles
N_CORES = 8
VS = V // N_CORES      # 4000 vocab rows per core
VCH = 500              # vocab cols per head matmul (<=512 psum bank)
NVC = VS // VCH        # 8 vocab chunks per core

_BUILD_CACHE = {}
_JIT_CACHE = {}


def _col(n, nt):
    """vst column for step n: b-major so head tiles are contiguous."""
    return (n % B) * (nt // B) + n // B


def _build(n_tok_tiles=NTT):
    nt = n_tok_tiles * P

    nc = bacc.Bacc(None, target_bir_lowering=False, debug=False)

    f32 = mybir.dt.float32
    bf16 = mybir.dt.bfloat16

    wt_d = nc.dram_tensor("wt", [P, DC * DC * P], bf16, kind="ExternalInput")
    xb_d = nc.dram_tensor("xb", [P, nt, DC], bf16, kind="ExternalInput")
    v0_d = nc.dram_tensor("v0m", [P, DC], bf16, kind="ExternalInput")
    hwt_d = nc.dram_tensor("hwt", [NVC, DC, P, VCH], bf16, kind="ExternalInput")
    out_d = nc.dram_tensor("out", [B, T, VS], bf16, kind="ExternalOutput")

    with tile.TileContext(nc) as tc:
        with (
            tc.tile_pool(name="const", bufs=1) as cpool,
            tc.tile_pool(name="outp", bufs=4) as opool,
            tc.tile_pool(name="zps", bufs=1, space="PSUM") as zpsum,
            tc.tile_pool(name="hps", bufs=2, space="PSUM") as hpsum,
        ):
            # ---- persistent SBUF state ----
            wt_sb = cpool.tile([P, DC * DC * P], bf16)    # W.T tiles (j,i)
            xb_sb = cpool.tile([P, nt, DC], bf16)         # x_n + b_r, d on partitions
            vst_sb = cpool.tile([P, DC, nt], bf16)        # vs.T store (b-major cols)
            v0_sb = cpool.tile([P, DC], bf16)
            vtmp_sb = cpool.tile([P, DC], bf16)           # pre-relu scratch
            hw_sb = [cpool.tile([P, DC, VCH], bf16, name=f"hw{nv}") for nv in range(NVC)]

            nc.sync.dma_start(wt_sb[:], wt_d[:])
            nc.sync.dma_start(v0_sb[:], v0_d[:])
            nc.sync.dma_start(xb_sb[:], xb_d[:])
            for nv in range(NVC):
                nc.sync.dma_start(hw_sb[nv][:], hwt_d[nv].rearrange("k p c -> p k c"))

            # ---- recurrence ----
            zp0 = zpsum.tile([P, DC], f32, name="zp0")
            zp1 = zpsum.tile([P, DC], f32, name="zp1")
            zps = [zp0, zp1]

            for n in range(nt):
                if n == 0:
                    vin = lambda j: v0_sb[:, j : j + 1]
                else:
                    cprev = _col(n - 1, nt)
                    vin = lambda j: vst_sb[:, j, ds(cprev, 1)]
                zp = zps[n % 2]
                for i in range(DC):
                    for j in range(DC):
                        nc.tensor.matmul(
                            zp[:, i : i + 1],
                            lhsT=wt_sb[:, ts(j * DC + i, P)],
                            rhs=vin(j),
                            start=(j == 0),
                            stop=(j == DC - 1),
                        )
                nc.vector.tensor_tensor(
                    out=vtmp_sb[:],
                    in0=zp[:],
                    in1=xb_sb[:, ds(n, 1), :],
                    op=mybir.AluOpType.add,
                )
                nc.vector.tensor_scalar_max(
                    vst_sb[:, :, ds(_col(n, nt), 1)], vtmp_sb[:], 0.0
                )

            # ---- head GEMM (vocab shard) ----
            for nv in range(NVC):
                for mt in range(n_tok_tiles):
                    hp = hpsum.tile([P, 512], f32, tag="hp")
                    for k in range(DC):
                        nc.tensor.matmul(
                            hp[:, :VCH],
                            lhsT=vst_sb[:, k, ts(mt, P)],
                            rhs=hw_sb[nv][:, k, :],
                            start=(k == 0),
                            stop=(k == DC - 1),
                        )
                    o_sb = opool.tile([P, VCH], bf16, tag="o")
                    nc.vector.tensor_copy(o_sb[:], hp[:, :VCH])
                    # vst cols [128mt, 128mt+128) = one contiguous (b, t) block
                    tpb = nt // B                      # t-positions per b
                    btiles = tpb // P                  # token tiles per b
                    b_idx = mt // btiles
                    t0 = (mt % btiles) * P
                    nc.sync.dma_start(
                        out_d[b_idx, ds(t0, P), ds(nv * VCH, VCH)],
                        o_sb[:],
                    )

    nc.compile()
    return nc


def _get_program(n_tok_tiles=NTT):
    if n_tok_tiles not in _BUILD_CACHE:
        _BUILD_CACHE[n_tok_tiles] = _build(n_tok_tiles)
    return _BUILD_CACHE[n_tok_tiles]


def _host_prep(input_seq, E, W_r, b_r, head_w, head_b, v0, n_tok_tiles=NTT):
    nt = n_tok_tiles * P
    tokens = np.ascontiguousarray(np.asarray(input_seq).T).reshape(-1)[:nt]
    E = np.asarray(E, np.float32)
    W = np.asarray(W_r, np.float32)
    b_r = np.asarray(b_r, np.float32)
    v0 = np.asarray(v0, np.float32)

    xsb = E[tokens] + b_r[None, :]                     # [nt, D] f32
    xb = np.ascontiguousarray(
        xsb.reshape(nt, DC, P).transpose(2, 0, 1)
    ).astype(BF16)                                     # [P, nt, DC]

    wt = np.ascontiguousarray(
        W.reshape(DC, P, DC, P).transpose(3, 2, 0, 1)
    ).reshape(P, DC * DC * P).astype(BF16)
    v0m = np.ascontiguousarray(v0.reshape(DC, P).T).astype(BF16)

    hw_bf = np.asarray(head_w, np.float32).astype(BF16)  # [V, D] bf16
    in_maps = []
    for c in range(N_CORES):
        hw_c = hw_bf[c * VS : (c + 1) * VS]            # [4000, 1024] bf16
        hwt_c = np.ascontiguousarray(
            hw_c.reshape(NVC, VCH, DC, P).transpose(0, 2, 3, 1)
        )
        in_maps.append({"wt": wt, "xb": xb, "v0m": v0m, "hwt": hwt_c})
    return in_maps


def _get_jitted(nc):
    """Cached shard_map-jitted executor for the bass program — the same
    lowering run_bass_kernel_spmd's axon path builds per call, built once.
    Donated zero output buffers are replaced by persistent device-resident
    arrays: the kernel writes every element of its output, and without
    donation the zero buffers survive across calls."""
    key = id(nc)
    if key in _JIT_CACHE:
        return _JIT_CACHE[key]

    import jax
    from jax.sharding import Mesh, PartitionSpec, NamedSharding
    from jax.experimental.shard_map import shard_map
    from concourse.bass2jax import (
        _bass_exec_p,
        install_neuronx_cc_hook,
        partition_id_tensor,
    )

    install_neuronx_cc_hook()

    partition_name = nc.partition_id_tensor.name if nc.partition_id_tensor else None
    in_names, out_names, out_avals = [], [], []
    for alloc in nc.m.functions[0].allocations:
        if not isinstance(alloc, mybir.MemoryLocationSet):
            continue
        name = alloc.memorylocations[0].name
        if alloc.kind == "ExternalInput":
            if name != partition_name:
                in_names.append(name)
        elif alloc.kind == "ExternalOutput":
            out_names.append(name)
            out_avals.append(
                jax.core.ShapedArray(
                    tuple(alloc.tensor_shape), mybir.dt.np(alloc.dtype)
                )
            )
    n_params, n_outs = len(in_names), len(out_avals)
    in_names_full = in_names + out_names
    if partition_name is not None:
        in_names_full.append(partition_name)

    def _body(*args):
        operands = list(args)
        if partition_name is not None:
            operands.append(partition_id_tensor())
        return tuple(
            _bass_exec_p.bind(
                *operands,
                out_avals=tuple(out_avals),
                in_names=tuple(in_names_full),
                out_names=tuple(out_names),
                lowering_input_output_aliases=(),
                sim_require_finite=True,
                sim_require_nnan=True,
                nc=nc,
            )
        )

    devices = jax.devices()[:N_CORES]
    assert len(devices) == N_CORES, f"need {N_CORES} cores, got {len(devices)}"
    mesh = Mesh(np.asarray(devices), ("core",))
    sharding = NamedSharding(mesh, PartitionSpec("core"))
    jitted = jax.jit(
        shard_map(
            _body,
            mesh=mesh,
            in_specs=(PartitionSpec("core"),) * (n_params + n_outs),
            out_specs=(PartitionSpec("core"),) * n_outs,
            check_rep=False,
        ),
        keep_unused=True,
    )
    zeros = [
        jax.device_put(
            np.zeros((N_CORES * a.shape[0], *a.shape[1:]), a.dtype), sharding
        )
        for a in out_avals
    ]
    jax.block_until_ready(zeros)
    entry = (jitted, in_names, out_names, out_avals, zeros)
    _JIT_CACHE[key] = entry
    return entry


def _run_fast(nc, in_maps):
    """Execute on cores 0-7; returns per-core output dicts."""
    jitted, in_names, out_names, out_avals, zeros = _get_jitted(nc)
    concat_in = [
        np.concatenate([np.asarray(in_maps[c][n]) for c in range(N_CORES)], axis=0)
        for n in in_names
    ]
    out_arrs = jitted(*concat_in, *zeros)
    return [
        {
            name: np.asarray(out_arrs[i]).reshape(N_CORES, *out_avals[i].shape)[c]
            for i, name in enumerate(out_names)
        }
        for c in range(N_CORES)
    ]


def run(inputs, n_tok_tiles=NTT, trace=False, tmpdir=None):
    """Run on hardware; returns (logits [B, T, V] f32, results).

    trace=False: cached-jit fast path; the second element is the list of
    per-core output dicts. trace=True: run_bass_kernel_spmd with NTFF
    profiling; the second element is the BassKernelResults."""
    nc = _get_program(n_tok_tiles)
    in_maps = _host_prep(**inputs, n_tok_tiles=n_tok_tiles)
    if trace:
        br = run_bass_kernel_spmd(
            nc,
            in_maps,
            core_ids=list(range(N_CORES)),
            trace=True,
            tmpdir=tmpdir,
        )
        results = br.results
    else:
        br = results = _run_fast(nc, in_maps)
    shards = [r["out"] for r in results]               # each [B, T, VS] bf16
    logits = np.concatenate(shards, axis=2).astype(np.float32)
    logits += np.asarray(inputs["head_b"], np.float32)[None, None, :]
    return logits, br


def kernel(input_seq, E, W_r, b_r, head_w, head_b, v0):
    inputs = dict(
        input_seq=input_seq, E=E, W_r=W_r, b_r=b_r,
        head_w=head_w, head_b=head_b, v0=v0,
    )
    logits, _ = run(inputs)
    return logits
